# revision 33
# baseline (speedup 1.0000x reference)
"""GNN message-passing (GENConv-style, 2 layers x 2 link types) on 8 trn2 cores.

Sharding: node-contiguous. Core c owns nodes [c*N/8, (c+1)*N/8). Since
dst(e) = e // K, core c also owns edges [c*E/8, (c+1)*E/8) for every link
type, and each node's K=16 edge slots are contiguous. The only remote data
dependency is the src-side gather x[src[e]], served from a replicated full
copy of x in each core's DRAM (layer 0: kernel input; layer 1: AllGather).

Math (per conv, per node n, channel h):
  msg_k = relu(x[src_k] + ea_k * We_h) + eps           (k = 0..15 slots)
  agg   = sum_k softmax_k(msg)*msg  = (S2 + eps*S1)/S1 with
          S1 = sum_k exp(msg_k'), S2 = sum_k exp(msg_k')*relu_k
  computed UNNORMALIZED (no max subtraction): msg' >= 0 and bounded well
  below 88, so exp never overflows/underflows in f32.
  Invalid slots (nbr < 0) are filled host-side with slot 0's (src, ea), so
  they contribute exactly z_n copies of slot 0's E/P, subtracted exactly via
  S1 -= z_n*E_0, S2 -= z_n*P_0 (slot 0 is always valid by construction).
  out = agg + x_own;  mlp: W1 -> BN(eval) -> relu -> W2; sum over link
  types; LeakyReLU(0.01) between layers.
"""

import os
import sys

import numpy as np

for _p in ("/opt/trn_rl_repo", os.path.expanduser("~/.axon_site/_ro/trn_rl_repo")):
    if os.path.isdir(_p) and _p not in sys.path:
        sys.path.insert(0, _p)

import concourse.bass as bass
import concourse.mybir as mybir
from concourse import bacc, library_config, tile
from concourse.bass_utils import run_bass_kernel_spmd

N = 32768
K = 16
H = 128
R = 2
L = 2
NCORES = 8
EPS_MSG = 1e-7
BN_EPS = 1e-5

f32 = mybir.dt.float32
i16 = mybir.dt.int16
AL = mybir.AluOpType
AF = mybir.ActivationFunctionType
AX = mybir.AxisListType


def build_program(n_nodes: int, n_cores: int, gather_queues: int = 2, reps: int = 1):
    """One SPMD bass program; per-core data differences come via inputs.

    reps > 1 repeats the whole network (benchmarking only)."""
    npc = n_nodes // n_cores          # nodes per core
    nt = npc // 128                   # 128-node tiles per core
    nc = bacc.Bacc("TRN2", num_devices=n_cores, num_swdge_queues=gather_queues)

    xfull = nc.declare_dram_parameter("xfull", [n_nodes, H], f32, isOutput=False)
    xown = nc.declare_dram_parameter("xown", [npc, H], f32, isOutput=False)
    idx16 = nc.declare_dram_parameter("idx16", [128, R * nt * 128], i16, isOutput=False)
    eaTk = nc.declare_dram_parameter("eaTk", [128, R * nt * 128], f32, isOutput=False)
    znegP = nc.declare_dram_parameter("zneg", [128, R * nt], f32, isOutput=False)
    weBD = nc.declare_dram_parameter("weBD", [128, L * R * K * H], f32, isOutput=False)
    w1T = nc.declare_dram_parameter("w1T", [128, L * R * 2 * H], f32, isOutput=False)
    w2T = nc.declare_dram_parameter("w2T", [128, L * R * 2 * H], f32, isOutput=False)
    bnS = nc.declare_dram_parameter("bnS", [128, L * R * 2], f32, isOutput=False)
    bnB = nc.declare_dram_parameter("bnB", [128, L * R * 2], f32, isOutput=False)
    eye = nc.declare_dram_parameter("eye", [128, 128], f32, isOutput=False)
    out = nc.declare_dram_parameter("out", [npc, H], f32, isOutput=True)

    h1own = nc.dram_tensor("h1own", [npc, H], f32)
    h1full = nc.dram_tensor("h1full", [n_nodes, H], f32)

    with tile.TileContext(nc) as tc:
        with (
            tc.tile_pool(name="const", bufs=1) as cp,
            tc.tile_pool(name="big", bufs=2) as bp,
            tc.tile_pool(name="node", bufs=3) as sp,
            tc.tile_pool(name="mlp", bufs=3) as mp,
            tc.tile_pool(name="ps", bufs=1, space="PSUM") as pp,
            tc.tile_pool(name="pst", bufs=4, space="PSUM") as pp2,
        ):
            idx_sb = cp.tile([128, R * nt * 128], i16)
            nc.sync.dma_start(idx_sb[:], idx16[:])
            ea_sb = cp.tile([128, R * nt * 128], f32)
            nc.sync.dma_start(ea_sb[:], eaTk[:])
            zn_sb = cp.tile([128, R * nt], f32)
            nc.sync.dma_start(zn_sb[:], znegP[:])
            we_sb = cp.tile([128, L * R * K * H], f32)
            nc.sync.dma_start(we_sb[:], weBD[:])
            w1_sb = cp.tile([128, L * R * 2 * H], f32)
            nc.sync.dma_start(w1_sb[:], w1T[:])
            w2_sb = cp.tile([128, L * R * 2 * H], f32)
            nc.sync.dma_start(w2_sb[:], w2T[:])
            bs_sb = cp.tile([128, L * R * 2], f32)
            nc.sync.dma_start(bs_sb[:], bnS[:])
            bb_sb = cp.tile([128, L * R * 2], f32)
            nc.sync.dma_start(bb_sb[:], bnB[:])
            eye_sb = cp.tile([128, 128], f32)
            nc.sync.dma_start(eye_sb[:], eye[:])
            epsb = cp.tile([128, 1], f32)
            nc.gpsimd.memset(epsb[:], EPS_MSG)

            gq = 0
            for layer in [l for _ in range(reps) for l in range(L)]:
                table = xfull if layer == 0 else h1full
                ownsrc = xown if layer == 0 else h1own
                dest = h1own if layer == 0 else out
                for t in range(nt):
                    xo = sp.tile([128, H], f32, tag="xo")
                    nc.sync.dma_start(xo[:], ownsrc[t * 128 : (t + 1) * 128, :])
                    y_ps = None
                    for r in range(R):
                        col = r * nt + t
                        lr = layer * R + r
                        G = bp.tile([128, K, H], f32, tag="G")
                        nc.gpsimd.dma_gather(
                            G[:],
                            table[:],
                            idx_sb[:, col * 128 : (col + 1) * 128],
                            num_idxs=K * 128,
                            num_idxs_reg=K * 128,
                            elem_size=H,
                            single_packet=False,
                            queue_num=gq,
                        )
                        gq = (gq + 1) % gather_queues

                        # t = G + ea (x) We on the PE: block-diag(We) matmul
                        # accumulated with an identity passthrough of G.
                        RT = bp.tile([128, K, H], f32, tag="RT")
                        for q in range(4):
                            ko = q * (K // 4)
                            t_ps = pp2.tile([128, K // 4, H], f32, tag="tps")
                            nc.tensor.matmul(
                                t_ps[:],
                                ea_sb[32 * q : 32 * q + K, col * 128 : (col + 1) * 128],
                                we_sb[
                                    32 * q : 32 * q + K,
                                    lr * K * H + ko * H : lr * K * H + (ko + K // 4) * H,
                                ],
                                start=True,
                                stop=False,
                                tile_position=(32 * q, 0),
                            )
                            nc.tensor.matmul(
                                t_ps[:],
                                eye_sb[:],
                                G[:, ko : ko + K // 4, :],
                                start=False,
                                stop=True,
                            )
                            nc.scalar.activation(
                                RT[:, ko : ko + K // 4, :],
                                t_ps[:],
                                AF.Relu,
                            )
                        E = bp.tile([128, K, H], f32, tag="E")
                        nc.scalar.activation(E[:], RT[:], AF.Exp, bias=epsb[:])
                        P = bp.tile([128, K, H], f32, tag="P")
                        nc.vector.tensor_tensor(P[:], E[:], RT[:], AL.mult)

                        S1 = sp.tile([128, H], f32, tag="S1")
                        nc.vector.tensor_reduce(
                            S1[:], E[:].rearrange("p k h -> p h k"), AX.X, AL.add
                        )
                        S2 = sp.tile([128, H], f32, tag="S2")
                        nc.vector.tensor_reduce(
                            S2[:], P[:].rearrange("p k h -> p h k"), AX.X, AL.add
                        )
                        # remove the z_n duplicated copies of slot 0
                        S1c = sp.tile([128, H], f32, tag="S1c")
                        nc.vector.scalar_tensor_tensor(
                            S1c[:], E[:, 0, :], zn_sb[:, col : col + 1], S1[:],
                            AL.mult, AL.add,
                        )
                        S2c = sp.tile([128, H], f32, tag="S2c")
                        nc.vector.scalar_tensor_tensor(
                            S2c[:], P[:, 0, :], zn_sb[:, col : col + 1], S2[:],
                            AL.mult, AL.add,
                        )
                        rcp = sp.tile([128, H], f32, tag="rcp")
                        nc.vector.reciprocal(rcp[:], S1c[:])
                        agg = sp.tile([128, H], f32, tag="agg")
                        nc.vector.tensor_tensor(agg[:], S2c[:], rcp[:], AL.mult)
                        # out = (agg + eps) + x_own   (the +eps term: msg = relu + eps)
                        ot = sp.tile([128, H], f32, tag="ot")
                        nc.vector.scalar_tensor_tensor(
                            ot[:], agg[:], float(EPS_MSG), xo[:], AL.add, AL.add
                        )

                        # ---- MLP ----
                        otT_ps = pp.tile([128, 128], f32, tag="tr")
                        nc.tensor.transpose(otT_ps[:], ot[:], eye_sb[:])
                        otT = mp.tile([128, 128], f32, tag="otT")
                        nc.scalar.copy(otT[:], otT_ps[:])
                        h1_ps = pp.tile([128, 2, 128], f32, tag="h1p")
                        for hf in range(2):
                            nc.tensor.matmul(
                                h1_ps[:, hf, :],
                                w1_sb[:, lr * 2 * H + hf * H : lr * 2 * H + (hf + 1) * H],
                                otT[:],
                                start=True,
                                stop=True,
                            )
                        h2 = []
                        for hf in range(2):
                            hh = mp.tile([128, 128], f32, tag=f"h2{hf}")
                            nc.scalar.activation(
                                hh[:],
                                h1_ps[:, hf, :],
                                AF.Relu,
                                bias=bb_sb[:, lr * 2 + hf : lr * 2 + hf + 1],
                                scale=bs_sb[:, lr * 2 + hf : lr * 2 + hf + 1],
                            )
                            h2.append(hh)
                        if y_ps is None:
                            y_ps = pp.tile([128, 128], f32, tag="yp")
                        for hf in range(2):
                            nc.tensor.matmul(
                                y_ps[:],
                                w2_sb[:, lr * 2 * H + hf * H : lr * 2 * H + (hf + 1) * H],
                                h2[hf][:],
                                start=(r == 0 and hf == 0),
                                stop=(r == 1 and hf == 1),
                            )

                    # finalize tile: optional LeakyReLU, transpose back, store
                    fin = sp.tile([128, 128], f32, tag="fin")
                    if layer < L - 1:
                        ycp = sp.tile([128, 128], f32, tag="ycp")
                        nc.scalar.copy(ycp[:], y_ps[:])
                        nc.vector.scalar_tensor_tensor(
                            fin[:], ycp[:], 0.01, ycp[:], AL.mult, AL.max
                        )
                    else:
                        nc.scalar.copy(fin[:], y_ps[:])
                    back_ps = pp.tile([128, 128], f32, tag="tr")
                    nc.tensor.transpose(back_ps[:], fin[:], eye_sb[:])
                    hrow = sp.tile([128, 128], f32, tag="hrow")
                    nc.scalar.copy(hrow[:], back_ps[:])
                    nc.sync.dma_start(dest[t * 128 : (t + 1) * 128, :], hrow[:])

                if layer == 0:
                    nc.gpsimd.collective_compute(
                        "AllGather",
                        AL.bypass,
                        replica_groups=[list(range(n_cores))],
                        ins=[h1own[:].opt()],
                        outs=[h1full[:].opt()],
                    )
    nc.finalize()
    return nc


def preprocess(x, edge_inds, edge_attrs, nbrs, W_edge, W1, bn_gamma, bn_beta,
               bn_mean, bn_var, W2, n_nodes=N, n_cores=NCORES):
    """Build per-core input maps (all numpy, f32/i16)."""
    npc = n_nodes // n_cores
    nt = npc // 128
    epc = npc * K

    x = np.ascontiguousarray(np.asarray(x, np.float32))
    src = np.asarray(edge_inds, np.int64)[:, 0, :]          # [R, E]
    ea = np.asarray(edge_attrs, np.float32)[:, :, 0]        # [R, E]
    valid = np.asarray(nbrs) >= 0                           # [R, n_nodes, K]

    We = np.asarray(W_edge, np.float32)[:, :, :, 0]         # [L, R, H]
    W1 = np.asarray(W1, np.float32)                         # [L, R, 2H, H]
    W2 = np.asarray(W2, np.float32)                         # [L, R, H, 2H]
    g = np.asarray(bn_gamma, np.float32)
    b = np.asarray(bn_beta, np.float32)
    m = np.asarray(bn_mean, np.float32)
    v = np.asarray(bn_var, np.float32)
    s = (g / np.sqrt(v + np.float32(BN_EPS))).astype(np.float32)   # [L, R, 2H]
    sh = (b - m * s).astype(np.float32)

    # shared (per-core identical) weight blocks
    # block-diag We for the PE rank-1 construction: weBD[k', lr*K*H + k*H + h],
    # replicated into partition groups 32q..32q+15 for PE row-tiling
    weBD1 = np.zeros((K, L * R, K, H), np.float32)
    for k in range(K):
        weBD1[k, :, k, :] = We.reshape(L * R, H)
    weBD1 = weBD1.reshape(K, L * R * K * H)
    weBD = np.zeros((128, L * R * K * H), np.float32)
    for q in range(4):
        weBD[32 * q : 32 * q + K] = weBD1
    w1T = W1.transpose(0, 1, 3, 2).reshape(L * R, H, 2 * H)         # [lr, h, o]
    w1T = w1T.transpose(1, 0, 2).reshape(H, L * R * 2 * H).copy()
    w2T = W2.transpose(0, 1, 3, 2).reshape(L * R, 2 * H, H)         # [lr, c, o2]
    w2T = (
        w2T.reshape(L * R, 2, H, H)                                  # [lr, chalf, c128, o2]
        .transpose(2, 0, 1, 3)
        .reshape(H, L * R * 2 * H)
        .copy()
    )
    bnS = s.reshape(L * R, 2, H).transpose(2, 0, 1).reshape(128, L * R * 2).copy()
    bnB = sh.reshape(L * R, 2, H).transpose(2, 0, 1).reshape(128, L * R * 2).copy()
    eye = np.eye(128, dtype=np.float32)

    in_maps = []
    for c in range(n_cores):
        n0 = c * npc
        e0 = c * epc
        src_c = src[:, e0 : e0 + epc].reshape(R, npc, K)
        ea_c = ea[:, e0 : e0 + epc].reshape(R, npc, K)
        val_c = valid[:, n0 : n0 + npc, :]                  # [R, npc, K]
        src_eff = np.where(val_c, src_c, src_c[:, :, 0:1])
        ea_eff = np.where(val_c, ea_c, ea_c[:, :, 0:1]).astype(np.float32)
        zneg = -(K - val_c.sum(axis=2)).astype(np.float32)  # [R, npc]

        # gather index layout: per tile, logical i = k*128 + p -> (node p, slot k);
        # wrapped [16, ni/16] then replicated to 128 partitions
        st = src_eff.reshape(R, nt, 128, K).transpose(0, 1, 3, 2)   # [R, nt, K, 128]
        logical = st.reshape(R, nt, K * 128)
        wrapped = logical.reshape(R, nt, 128, 16).transpose(0, 1, 3, 2)  # [R,nt,16,128]
        idx = np.broadcast_to(
            wrapped[:, :, None, :, :], (R, nt, 8, 16, 128)
        ).reshape(R, nt, 128, 128)
        idx16 = idx.transpose(2, 0, 1, 3).reshape(128, R * nt * 128)
        idx16 = np.ascontiguousarray(idx16, dtype=np.int16)

        # eaTk[32q + k, (r*nt+t)*128 + n] = ea_eff[r, t*128+n, k] (4 replicas)
        eaTk1 = np.ascontiguousarray(
            ea_eff.reshape(R, nt, 128, K).transpose(3, 0, 1, 2).reshape(K, R * nt * 128)
        )
        eaTk = np.zeros((128, R * nt * 128), np.float32)
        for q in range(4):
            eaTk[32 * q : 32 * q + K] = eaTk1
        znegP = np.ascontiguousarray(
            zneg.reshape(R, nt, 128).transpose(2, 0, 1).reshape(128, R * nt)
        )

        in_maps.append(
            {
                "xfull": x,
                "xown": np.ascontiguousarray(x[n0 : n0 + npc]),
                "idx16": idx16,
                "eaTk": eaTk,
                "zneg": znegP,
                "weBD": weBD,
                "w1T": w1T,
                "w2T": w2T,
                "bnS": bnS,
                "bnB": bnB,
                "eye": eye,
            }
        )
    return in_maps


_PROG = {}


def kernel(**inputs) -> np.ndarray:
    n_nodes = inputs["x"].shape[0]
    n_cores = NCORES
    key = (n_nodes, n_cores)
    if key not in _PROG:
        _PROG[key] = build_program(n_nodes, n_cores)
    nc = _PROG[key]
    in_maps = preprocess(**inputs, n_nodes=n_nodes, n_cores=n_cores)
    res = run_bass_kernel_spmd(nc, in_maps, list(range(n_cores)))
    return np.concatenate([res.results[c]["out"] for c in range(n_cores)], axis=0)
